# revision 41
# baseline (speedup 1.0000x reference)
"""Trainium2 Bass kernel for nn_BoltzmannMachine: one sequential Gibbs sweep
over N=8192 units (order `perm`), distributed over 8 NeuronCores.

Algorithm (exact, validated vs the jax reference):
  sigmoid(s/T) >= u  <=>  s >= T*logit(u); thresholds are precomputed on
  device from u. Clamped units (half of them) never change and need no
  field evaluation, so only the NF=4096 free steps are processed: 8 row
  blocks of 512. Clamped COLUMNS contribute a purely static s0-signed
  row-sum folded into each block's margin. Within a block the decision
  bits satisfy a strictly-lower-triangular fixed point solved by Jacobi
  iteration on PE+DVE (R_ROUNDS covers the empirical worst case, with a
  convergence flag + retry ladder for safety). Block-start margins
  accumulate per-core partial sums over each core's 1024-column stripe
  (512 free + 512 clamped cols, premultiplied by s0 signs on the host -
  exact +-1 flips); an AllGather combines them.

Pipeline highlights:
  - All three AllGathers fire with >= 2 ticks of cover: {0,1} and
    {2..5} during prefill (the quad's past columns all arrive via the
    on-PE H1..H5 corrections, depth (m-2)%4+2), {6,7} at tick 4 (its
    streamed pieces for cols 0..3 land during ticks 0..3).
  - H-correction levels >= 2 run as a closed PSUM accumulation group in
    the PREVIOUS tick's tail (their deltas are already final), so only
    H1's 16 matmuls sit between a block's delta and the next Jacobi
    solve. PSUM groups are never left open across other matmuls (the
    hardware semantics forbid interleaving).
  - Per-block Jacobi round counts follow ROUNDS_SCHED (empirical worst
    case + safety margin; a convergence flag + host margin-replay
    verification + retry ladder guarantee correctness for any input).
  - The streamed block-column pieces read a per-partition-contiguous
    host layout (wpiece) so every DMA run is >= 512B (full-rate).
  - V@h0 per block precomputed on PE ahead of time; each Jacobi round
    is 10 matmuls + one is_ge vector op against a negated threshold.
  - DMA load split across both HWDGE queues (SP + Activation); the
    collectives keep the Pool queue to themselves.

Host does data movement on w (permutation gather / re-layout / sign
flips) plus O(N) precompute; all O(N^2) FLOPs and the sequential
resolution run on device.
"""
import numpy as np

N = 8192
NF = 4096          # free steps (clamping_degree==0); harness input has 4096
B = 512
CORES = 8
F = B // 128
NBLK = NF // B     # 8 row blocks / free col blocks
RT = NF // 128     # 32 row tiles
SW = N // CORES    # 1024 stripe cols per core: 512 free + 512 clamped
BW = B // CORES    # 64 free cols per block per core
R_ROUNDS = 10

NPAIR = F * (F + 1) // 2


def _tile_order(vec):
    return np.ascontiguousarray(vec.reshape(RT, 128).T)


def _depth(m):
    """number of H corrections for block m under the AG grouping"""
    if m < 2:
        return m
    return (m - 2) % 4 + 2


ROUNDS_SCHED = (6, 8, 7, 8, 11, 6, 6, 5)   # empirical per-block + safety 3


def _build_nc(R=R_ROUNDS, timing_no_cc=False, body_reps=1, sched=None,
              engines_v2=False, psh_early=True):
    import concourse.bacc as bacc
    import concourse.bass as bass
    import concourse.mybir as mybir
    from concourse.tile import TileContext

    f32 = mybir.dt.float32
    AO = mybir.AluOpType

    nc = bacc.Bacc("TRN2", target_bir_lowering=False, debug=False,
                   num_devices=CORES)

    wstripe = nc.declare_dram_parameter("wstripe", [NF, SW], f32,
                                        isOutput=False)
    wpiece = nc.declare_dram_parameter("wpiece", [128, NBLK * RT * BW], f32,
                                       isOutput=False)
    vpack = nc.declare_dram_parameter("vpack", [128, NBLK * NPAIR * 128], f32,
                                      isOutput=False)
    h1pack = nc.declare_dram_parameter("h1pack", [128, NBLK * F * F * 128],
                                       f32, isOutput=False)
    h2pack = nc.declare_dram_parameter("h2pack", [128, NBLK * F * F * 128],
                                       f32, isOutput=False)
    h3pack = nc.declare_dram_parameter("h3pack", [128, NBLK * F * F * 128],
                                       f32, isOutput=False)
    h4pack = nc.declare_dram_parameter("h4pack", [128, NBLK * F * F * 128],
                                       f32, isOutput=False)
    h5pack = nc.declare_dram_parameter("h5pack", [128, NBLK * F * F * 128],
                                       f32, isOutput=False)
    u_t = nc.declare_dram_parameter("u_t", [128, RT], f32, isOutput=False)
    q_t = nc.declare_dram_parameter("q_t", [128, RT], f32, isOutput=False)
    h0_t = nc.declare_dram_parameter("h0_t", [128, RT], f32, isOutput=False)
    s0_t = nc.declare_dram_parameter("s0_t", [128, RT], f32, isOutput=False)
    t_rep = nc.declare_dram_parameter("t_rep", [128, 1], f32, isOutput=False)
    out_d = nc.declare_dram_parameter("out_vals", [128, RT], f32,
                                      isOutput=True)
    flg_d = nc.declare_dram_parameter("out_flags", [128, NBLK], f32,
                                      isOutput=True)

    with TileContext(nc) as tc:
        with (
            tc.tile_pool(name="res", bufs=1) as res,
            tc.tile_pool(name="wbig", bufs=4) as wbig,
            tc.tile_pool(name="prod", bufs=2) as prodp,
            tc.tile_pool(name="pk", bufs=3) as pkp,
            tc.tile_pool(name="sm", bufs=3) as smp,
            tc.tile_pool(name="ytp", bufs=2) as ytpool,
            tc.tile_pool(name="ps", bufs=2, space=bass.MemorySpace.PSUM) as psp,
            tc.tile_pool(name="psv2", bufs=1,
                         space=bass.MemorySpace.PSUM) as psvp,
            tc.tile_pool(name="psh2", bufs=1,
                         space=bass.MemorySpace.PSUM) as pshp,
            tc.tile_pool(name="psvb", bufs=2,
                         space=bass.MemorySpace.PSUM) as psvbp,
            tc.tile_pool(name="cin", bufs=3, space="DRAM") as cin,
            tc.tile_pool(name="cout", bufs=3, space="DRAM") as cout,
        ):
            cid = nc.vector.partition_id()
            cid_be = nc.gpsimd.partition_id() if engines_v2 else cid
            # engine roles: DVE (`se`) runs the latency-critical small ops;
            # in v2 the big streaming reduces move to the Pool engine so
            # they never head-of-line-block a Jacobi round in the DVE
            # queue. ACT+SP keep the bulk DMA triggers.
            se = nc.vector
            be = nc.gpsimd if engines_v2 else nc.vector
            bgq = (nc.sync, nc.scalar)

            # ---------- resident tiles ----------
            acc = res.tile([128, RT], f32)
            th = res.tile([128, RT], f32)
            qt = res.tile([128, RT], f32)
            h0 = res.tile([128, RT], f32)
            s0t = res.tile([128, RT], f32)
            outv = res.tile([128, RT], f32)
            flags = res.tile([128, NBLK], f32)
            bits = res.tile([128, F], f32)
            mb0 = res.tile([128, F], f32)
            nm = res.tile([128, F], f32)
            bprev = res.tile([128, F], f32)
            trep = res.tile([128, 1], f32)
            ones1 = res.tile([1, 128], f32)
            fs = res.tile([128, F], f32)
            fsrow = res.tile([1, B], f32)
            d0 = res.tile([128, F], f32)
            d1 = res.tile([128, F], f32)
            d2 = res.tile([128, F], f32)
            d3 = res.tile([128, F], f32)
            d4 = res.tile([128, F], f32)
            d5 = res.tile([128, F], f32)
            dtiles = [d0, d1, d2, d3, d4, d5]

            for rep in range(body_reps):
                nc.vector.memset(acc[:, :], 0.0)
                nc.vector.memset(flags[:, :], 0.0)
                for dt_ in dtiles:
                    nc.vector.memset(dt_[:, :], 0.0)
                nc.vector.memset(ones1[:, :], 1.0)

                # ---------- load vectors ----------
                utile = smp.tile([128, RT], f32, tag="uload")
                nc.sync.dma_start(out=utile[:, :], in_=u_t[:, :])
                nc.scalar.dma_start(out=h0[:, :], in_=h0_t[:, :])
                nc.scalar.dma_start(out=s0t[:, :], in_=s0_t[:, :])
                nc.scalar.dma_start(out=qt[:, :], in_=q_t[:, :])
                nc.sync.dma_start(out=trep[:, :], in_=t_rep[:, :])

                # th = T * (ln(u) - ln(1-u))
                lu = smp.tile([128, RT], f32, tag="lu")
                om = smp.tile([128, RT], f32, tag="om")
                nc.scalar.activation(lu[:, :], utile[:, :],
                                     mybir.ActivationFunctionType.Ln)
                se.tensor_scalar(om[:, :], utile[:, :], -1.0, 1.0,
                                 AO.mult, AO.add)
                nc.scalar.activation(om[:, :], om[:, :],
                                     mybir.ActivationFunctionType.Ln)
                se.tensor_tensor(out=lu[:, :], in0=lu[:, :],
                                 in1=om[:, :], op=AO.subtract)
                se.tensor_scalar(th[:, :], lu[:, :], trep[:, 0:1],
                                 None, AO.mult)

                # ---------- helper: matvec contribution ----------
                qtoggle = [0]

                def piece(row_tile0, n_row_tiles, colL0, colW, vb_ap,
                          blk=None):
                    """acc[rows] += sum_cols wstripe(rows, cols) [* vb]"""
                    X = n_row_tiles
                    eng = bgq[qtoggle[0] % 2]
                    qtoggle[0] += 1
                    wt = wbig.tile([128, X * colW], f32, tag="wt")
                    if blk is not None:
                        # contiguous per-partition layout: >=512B runs
                        wpv = wpiece.ap().rearrange("p (k xt c) -> p k xt c",
                                                    k=NBLK, xt=RT)
                        xt0 = row_tile0
                        eng.dma_start(
                            out=wt[:, :].rearrange("p (xt c) -> p xt c", xt=X),
                            in_=wpv[:, blk, xt0:xt0 + X, :])
                    else:
                        wsv = wstripe.ap().rearrange("(xt p) c -> p xt c",
                                                     p=128)
                        eng.dma_start(
                            out=wt[:, :].rearrange("p (xt c) -> p xt c", xt=X),
                            in_=wsv[:, row_tile0:row_tile0 + X,
                                    colL0:colL0 + colW])
                    if vb_ap is not None:
                        pr = prodp.tile([128, X * colW], f32, tag="pr")
                        nc.vector.scalar_tensor_tensor(
                            out=pr[:, :].rearrange("p (xt c) -> p xt c", xt=X),
                            in0=wt[:, :].rearrange("p (xt c) -> p xt c", xt=X),
                            scalar=1.0,
                            in1=vb_ap.unsqueeze(1).to_broadcast(
                                (128, X, colW)),
                            op0=AO.mult, op1=AO.mult)
                        src = pr
                    else:
                        src = wt
                    red = smp.tile([128, X], f32, tag="red")
                    nc.vector.tensor_reduce(
                        out=red[:, :],
                        in_=src[:, :X * colW].rearrange("p (xt c) -> p xt c",
                                                        xt=X),
                        axis=mybir.AxisListType.X, op=AO.add)
                    be.tensor_tensor(
                        out=acc[:, row_tile0:row_tile0 + X],
                        in0=acc[:, row_tile0:row_tile0 + X],
                        in1=red[:, :], op=AO.add)

                def upper(m):
                    # static premultiplied s0 contribution (pure reduce):
                    # free columns >= m - depth(m) plus ALL clamped columns
                    # (the clamped 512 sit at stripe offset 512..1023, so
                    # the range [colL0, SW) covers both in one sweep)
                    colL0 = max(0, (m - _depth(m)) * BW)
                    c0 = colL0
                    while c0 < SW:
                        cw = min(512, SW - c0)
                        piece(F * m, F, c0, cw, None)
                        c0 += cw

                rg = [list(range(CORES))]
                outb = {}

                def trigger_ag_n(m, nb):
                    """AllGather for blocks {m .. m+nb-1}."""
                    ib = cin.tile([128, nb * F], f32, tag="ib", bufs=2)
                    ob = cout.tile([CORES * 128, nb * F], f32, tag="ob",
                                   bufs=2)
                    nc.sync.dma_start(out=ib[:, :],
                                      in_=acc[:, F * m:F * (m + nb)])
                    if timing_no_cc:
                        nc.sync.dma_start(out=ob[0:128, :], in_=ib[:, :])
                    else:
                        nc.gpsimd.collective_compute(
                            "AllGather", AO.bypass, replica_groups=rg,
                            ins=[ib[:, :].opt()], outs=[ob[:, :].opt()])
                    for i in range(nb):
                        outb[m + i] = (ob, i * F, nb)

                def qeng():
                    eng = bgq[qtoggle[0] % 2]
                    qtoggle[0] += 1
                    return eng

                def prefetch_packs(m):
                    vp = pkp.tile([128, NPAIR * 128], f32, tag="vp")
                    off = m * NPAIR * 128
                    qeng().dma_start(out=vp[:, :],
                                     in_=vpack[:, off:off + NPAIR * 128])
                    hsrc_d = {1: h1pack, 2: h2pack, 3: h3pack,
                              4: h4pack, 5: h5pack}
                    hps = []
                    off = m * F * F * 128
                    for k in range(1, _depth(m) + 1):
                        hk = pkp.tile([128, F * F * 128], f32, tag=f"hp{k}")
                        qeng().dma_start(
                            out=hk[:, :],
                            in_=hsrc_d[k][:, off:off + F * F * 128])
                        hps.append(hk)
                    return (vp, hps)

                def compute_vh0(m, vp):
                    ps = psvp.tile([128, F], f32, tag="vh0")
                    for mc in range(F):
                        for kc in range(mc + 1):
                            poff = (mc * (mc + 1) // 2 + kc) * 128
                            nc.tensor.matmul(
                                ps[:, mc:mc + 1],
                                vp[:, poff:poff + 128],
                                h0[:, F * m + kc:F * m + kc + 1],
                                start=(kc == 0), stop=(kc == mc))
                    sb_t = smp.tile([128, F], f32, tag="vh0s", bufs=3)
                    nc.vector.tensor_copy(sb_t[:, :], ps[:, :])
                    return sb_t

                # ---------- prefill ----------
                upper(0)
                upper(1)
                trigger_ag_n(0, 2)
                upper(2)
                upper(3)
                upper(4)
                upper(5)
                # quad {2..5} needs no streamed pieces (H-band covers all
                # its past cols), so its AG can fire as soon as the static
                # uppers are in -- two full ticks of latency cover
                trigger_ag_n(2, 4)

                packs = {0: prefetch_packs(0), 1: prefetch_packs(1)}
                vh0ps = {0: compute_vh0(0, packs[0][0]),
                         1: compute_vh0(1, packs[1][0])}

                # ---------- main tick loop ----------
                pshpend = {}

                def _psh_mms(tilep, hs, start0):
                    for mc in range(F):
                        for hi, (h, d) in enumerate(hs):
                            for kc in range(F):
                                nc.tensor.matmul(
                                    tilep[:, mc:mc + 1],
                                    h[:, (kc * F + mc) * 128:
                                      (kc * F + mc) * 128 + 128],
                                    d[:, kc:kc + 1],
                                    start=(start0 and hi == 0 and kc == 0),
                                    stop=(hi == len(hs) - 1 and
                                          kc == F - 1))

                def prep_psh_early(m):
                    """levels >=2 for block m as a CLOSED accumulation group
                    (their deltas are final before tick m-1 ends, so the PE
                    runs this during tick m-1's tail, off the critical
                    path). PSUM groups must not interleave with other
                    matmuls, so H1 goes to a second tile at tick m."""
                    hps = packs[m][1]
                    tile_e = None
                    if psh_early and len(hps) >= 2:
                        tile_e = pshp.tile([128, F], f32, tag="pshE")
                        hs = [(hps[k - 1], dtiles[(m - k) % 6])
                              for k in range(2, len(hps) + 1)]
                        _psh_mms(tile_e, hs, True)
                    pshpend[m] = tile_e

                def finish_psh(m):
                    """H1 (deltas of block m-1); with psh_early off, all
                    levels run here (old style)."""
                    tile_e = pshpend.pop(m)
                    hps = packs[m][1]
                    levels = [1] if tile_e is not None else \
                        list(range(1, len(hps) + 1))
                    hs = [(hps[k - 1], dtiles[(m - k) % 6]) for k in levels]
                    tile1 = pshp.tile([128, F], f32, tag="psh1")
                    _psh_mms(tile1, hs, True)
                    return (tile1, tile_e)

                for j in range(NBLK):
                    dcur = dtiles[j % 6]

                    # 0) fire the {6,7} AG at tick 4: its pieces (cols 0..3)
                    #    landed by the end of tick 3, giving 2 ticks cover
                    if j == 4 and NBLK == 8:
                        trigger_ag_n(6, 2)

                    # 1) close block j's H correction with the H1 matmuls
                    psh = finish_psh(j) if j >= 1 else None

                    # 3) gather AG result, reduce over cores
                    ob, ofs, nbw = outb[j]
                    if j == 0 or j % 4 == 2:
                        yt2 = ytpool.tile([128, nbw * F * CORES], f32,
                                          tag="yt2")
                        nc.sync.dma_start(
                            out=yt2[:, :].rearrange("p (f c) -> p f c",
                                                    c=CORES),
                            in_=ob[:, :].rearrange("(c p) f -> p f c", p=128))
                        ytcache = yt2
                    yv = smp.tile([128, F], f32, tag="yv")
                    se.tensor_reduce(
                        out=yv[:, :],
                        in_=ytcache[:, :].rearrange(
                            "p (f c) -> p f c", c=CORES)[:, ofs:ofs + F, :],
                        axis=mybir.AxisListType.X, op=AO.add)

                    # 4) mb0 = yv - th (+ psh); bits0 = [mb0 >= 0]
                    se.tensor_tensor(out=mb0[:, :], in0=yv[:, :],
                                     in1=th[:, F * j:F * (j + 1)],
                                     op=AO.subtract)
                    if psh is not None:
                        tile1, tile_e = psh
                        if tile_e is not None:
                            se.tensor_tensor(out=mb0[:, :], in0=mb0[:, :],
                                             in1=tile_e[:, :], op=AO.add)
                        se.tensor_tensor(out=mb0[:, :], in0=mb0[:, :],
                                         in1=tile1[:, :], op=AO.add)
                    se.tensor_scalar(bits[:, :], mb0[:, :], 0.0, None,
                                     AO.is_ge)
                    se.scalar_tensor_tensor(
                        out=nm[:, :], in0=vh0ps[j][:, :], scalar=-1.0,
                        in1=mb0[:, :], op0=AO.mult, op1=AO.subtract)

                    # 5) fixed-point rounds
                    vp = packs[j][0]
                    Rj = sched[j] if sched is not None else R
                    for r in range(1, Rj):
                        if r == Rj - 1:
                            se.tensor_copy(bprev[:, :], bits[:, :])
                        psv = psp.tile([128, F], f32, tag="psv")
                        for mc in range(F):
                            for kc in range(mc + 1):
                                poff = (mc * (mc + 1) // 2 + kc) * 128
                                nc.tensor.matmul(
                                    psv[:, mc:mc + 1],
                                    vp[:, poff:poff + 128],
                                    bits[:, kc:kc + 1],
                                    start=(kc == 0), stop=(kc == mc))
                        se.tensor_tensor(out=bits[:, :],
                                         in0=psv[:, :],
                                         in1=nm[:, :], op=AO.is_ge)

                    # 6) delta first (it unblocks tick j+1's H1 and the
                    #    flip-sign pieces), outv, then flags off-path
                    # delta = 2*bits - (1+s0)
                    se.scalar_tensor_tensor(
                        out=dcur[:, :], in0=bits[:, :], scalar=2.0,
                        in1=qt[:, F * j:F * (j + 1)],
                        op0=AO.mult, op1=AO.subtract)
                    se.tensor_tensor(out=outv[:, F * j:F * (j + 1)],
                                     in0=s0t[:, F * j:F * (j + 1)],
                                     in1=dcur[:, :], op=AO.add)

                    # ---- tail: prepare tick j+1 ----
                    # 7) early H levels (>=2) for block j+1 run on PE now
                    if j + 1 < NBLK:
                        prep_psh_early(j + 1)

                    # 8) flip-sign pieces for col j onto rows whose AG
                    #    snapshot is still in the future
                    sb = 4 * (j // 4) + 6
                    X = RT - F * sb
                    if X > 0:
                        se.tensor_tensor(
                            out=fs[:, :], in0=dcur[:, :],
                            in1=s0t[:, F * j:F * (j + 1)], op=AO.mult)
                        se.tensor_scalar(fs[:, :], fs[:, :], 1.0,
                                         None, AO.add)
                        nc.scalar.dma_start(
                            out=fsrow[0:1, :].rearrange("a (p f) -> a p f",
                                                        p=128),
                            in_=fs[:, :])
                        vbps = psvbp.tile([128, B], f32, tag="vb")
                        nc.tensor.matmul(vbps[:, :], ones1[:, :],
                                         fsrow[:, :], start=True, stop=True)
                        piece(F * sb, X, j * BW, BW,
                              vbps[:, bass.ts(cid, BW)], blk=j)

                    # 9) flags (convergence check, off the critical path)
                    dtmp = smp.tile([128, F], f32, tag="dtmp")
                    se.tensor_tensor(out=dtmp[:, :], in0=bits[:, :],
                                     in1=bprev[:, :], op=AO.subtract)
                    nc.vector.tensor_reduce(out=flags[:, j:j + 1],
                                            in_=dtmp[:, :],
                                            axis=mybir.AxisListType.X,
                                            op=AO.add,
                                            apply_absolute_value=True)

                    # 10) background: upper(j+6) (prefill issued 0..5)
                    if j + 6 < NBLK:
                        upper(j + 6)

                    # 11) background: prefetch packs + Vh0 two ticks ahead
                    if j + 2 < NBLK and (j + 2) not in packs:
                        packs[j + 2] = prefetch_packs(j + 2)
                        vh0ps[j + 2] = compute_vh0(j + 2, packs[j + 2][0])
                    packs.pop(j - 1, None)
                    vh0ps.pop(j - 1, None)

                nc.sync.dma_start(out=out_d[:, :], in_=outv[:, :])
                nc.sync.dma_start(out=flg_d[:, :], in_=flags[:, :])

    nc.compile()
    return nc


def _host_prep(w, initial_state, u, T, clamping_degree, perm):
    w = np.asarray(w, dtype=np.float32)
    s0 = np.asarray(initial_state, dtype=np.float32)
    u = np.asarray(u, dtype=np.float32)
    cd = np.asarray(clamping_degree)
    perm = np.asarray(perm).astype(np.int64)
    Tf = np.float32(T)

    free_step = cd[perm] == 0
    js = np.nonzero(free_step)[0]            # free step indices, in order
    cs = np.nonzero(~free_step)[0]           # clamped step indices
    jj = perm[js]                            # free unit ids, visit order
    ccu = perm[cs]                           # clamped unit ids

    s0f = s0[jj]
    uf = u[js]
    h0 = (-0.5 * (1.0 + s0f)).astype(np.float32)

    wr = w[jj]                               # [NF, N] rows in visit order

    jf, pt, ff = np.meshgrid(np.arange(NBLK), np.arange(16), np.arange(F),
                             indexing="ij")

    def stripe_free(c):
        # free-column visit indices owned by core c, stripe order
        return (jf * B + 128 * ff + 16 * c + pt).reshape(-1)

    # intra/inter block packs (free x free couplings)
    wff = w[np.ix_(jj, jj)]
    vpack = np.zeros((128, NBLK * NPAIR * 128), dtype=np.float32)
    h1pack = np.zeros((128, NBLK * F * F * 128), dtype=np.float32)
    h2pack = np.zeros((128, NBLK * F * F * 128), dtype=np.float32)
    h3pack = np.zeros((128, NBLK * F * F * 128), dtype=np.float32)
    h4pack = np.zeros((128, NBLK * F * F * 128), dtype=np.float32)
    h5pack = np.zeros((128, NBLK * F * F * 128), dtype=np.float32)
    tril = np.tril(np.ones((B, B), dtype=np.float32), -1)
    for j in range(NBLK):
        blk = wff[j * B:(j + 1) * B, j * B:(j + 1) * B]
        V = (blk * tril) * 2.0               # f2 = 2 (all free)
        hsrc = {}
        for k in range(1, _depth(j) + 1):
            hsrc[k] = wff[j * B:(j + 1) * B, (j - k) * B:(j - k + 1) * B]
        packs_ = {1: h1pack, 2: h2pack, 3: h3pack, 4: h4pack, 5: h5pack}
        for mc in range(F):
            for kc in range(F):
                colbase = (j * F * F + kc * F + mc) * 128
                for lvl, Hm in hsrc.items():
                    packs_[lvl][:, colbase:colbase + 128] = \
                        Hm[mc * 128:(mc + 1) * 128,
                           kc * 128:(kc + 1) * 128].T
            for kc in range(mc + 1):
                vbase = (j * NPAIR + mc * (mc + 1) // 2 + kc) * 128
                vpack[:, vbase:vbase + 128] = V[mc * 128:(mc + 1) * 128,
                                                kc * 128:(kc + 1) * 128].T

    common = {
        "vpack": vpack,
        "h1pack": h1pack,
        "h2pack": h2pack,
        "h3pack": h3pack,
        "h4pack": h4pack,
        "h5pack": h5pack,
        "u_t": _tile_order(uf),
        "q_t": _tile_order((1.0 + s0f).astype(np.float32)),
        "h0_t": _tile_order(h0),
        "s0_t": _tile_order(s0f),
        "t_rep": np.full((128, 1), Tf, dtype=np.float32),
    }
    in_maps = []
    ncl = len(ccu)
    for c in range(CORES):
        sf = stripe_free(c)
        # clamped columns owned by core c (even 512-way split)
        scl = ccu[(np.arange(ncl) % CORES) == c]
        # stripe = [512 free cols (visit order) | 512 clamped cols],
        # premultiplied by their s0 signs (exact +-1 flips)
        cols_units = np.concatenate([jj[sf], scl])
        sgn = s0[cols_units][None, :]
        wsm = np.ascontiguousarray(wr[:, cols_units] * sgn)
        m = dict(common)
        m["wstripe"] = wsm
        # per-partition-contiguous block-column layout for the bulk pieces
        # (free half only)
        m["wpiece"] = np.ascontiguousarray(
            wsm[:, :NBLK * BW].reshape(RT, 128, NBLK, BW)
            .transpose(1, 2, 0, 3).reshape(128, NBLK * RT * BW))
        in_maps.append(m)
    meta = {"perm": perm, "s0": s0, "jj": jj, "wr": wr,
            "th64": np.float64(T) * (np.log(np.float64(uf)) -
                                     np.log1p(-np.float64(uf)))}
    return in_maps, meta


def _verify_device_result(vals_pp, meta, tol=1e-3):
    """Replay the blockwise margins on host and confirm every device
    decision.  Decisions inside +-tol of the threshold are accepted (that
    is the irreducible fp32 ambiguity band); anything farther off means a
    corrupted run (e.g. a DMA race) and triggers a retry/fallback."""
    jj = meta["jj"]
    wr = meta["wr"]
    th = meta["th64"]
    if not np.isfinite(vals_pp).all() or \
            np.abs(np.abs(vals_pp) - 1.0).max() != 0.0:
        return False
    state = np.array(meta["s0"], dtype=np.float32, copy=True)
    for b in range(NBLK):
        sl = slice(b * B, (b + 1) * B)
        dots = (wr[sl].astype(np.float64) @ np.float64(state))
        margin = dots - th[sl]
        want = np.where(margin >= 0, np.float32(1.0), np.float32(-1.0))
        got = vals_pp[sl]
        bad = (want != got) & (np.abs(margin) > tol)
        if bad.any():
            return False
        state[jj[sl]] = got
    return True


_NC_CACHE = {}
LAST_RESULTS = None  # BassKernelResults of the final device run (for test.py)


def _get_nc(key, **kw):
    if key not in _NC_CACHE:
        _NC_CACHE[key] = _build_nc(**kw)
    return _NC_CACHE[key]


def default_nc(body_reps=1):
    return _get_nc(("sched", ROUNDS_SCHED, body_reps),
                   sched=ROUNDS_SCHED, body_reps=body_reps,
                   engines_v2=False)


def kernel(**inputs):
    global LAST_RESULTS
    from concourse.bass_utils import run_bass_kernel_spmd
    import os

    w = inputs["w"]
    perm = np.asarray(inputs["perm"]).astype(np.int64)
    cd = np.asarray(inputs["clamping_degree"])
    # fast path requires a true permutation and the expected free count
    is_perm = (np.sort(perm) == np.arange(N)).all()
    if not is_perm or int((cd[perm] == 0).sum()) != NF:
        return _reference_fallback(**inputs)

    in_maps, meta = _host_prep(**inputs)
    trace = os.environ.get("KERNEL_TRACE", "0") == "1"

    # retry ladder: per-block schedule first, then uniform deep rounds.
    # every candidate result is verified on the host (blockwise margin
    # replay) before being accepted, so a flaky run can never escape.
    ok = False
    for attempt in ("sched", 16, 64):
        if attempt == "sched":
            nc = default_nc()
        else:
            nc = _get_nc(("uniform", attempt), R=attempt)
        res = run_bass_kernel_spmd(nc, in_maps, core_ids=list(range(CORES)),
                                   trace=trace)
        LAST_RESULTS = res
        vals_t = res.results[0]["out_vals"]       # [128, RT] tile layout
        flags = res.results[0]["out_flags"]
        vals_pp = vals_t.T.reshape(-1)            # [NF] free-step order
        if float(np.abs(flags).sum()) == 0.0 and \
                _verify_device_result(vals_pp, meta):
            ok = True
            break
    if not ok:
        return _reference_fallback(**inputs)
    out = np.array(meta["s0"], dtype=np.float32, copy=True)
    out[meta["jj"]] = vals_pp
    return out


def _reference_fallback(w, initial_state, u, T, clamping_degree, perm):
    """Generic (repeat-tolerant) path: exact sequential numpy replay.

    Only used when `perm` is not a permutation or the clamping pattern
    differs from the expected harness inputs.
    """
    state = np.asarray(initial_state, dtype=np.float64).copy()
    w64 = np.asarray(w, dtype=np.float64)
    free = (np.asarray(clamping_degree) == 0)
    th = float(T) * (np.log(np.float64(u)) - np.log1p(-np.float64(u)))
    for t in range(len(perm)):
        j = int(perm[t])
        if free[j]:
            s = w64[j] @ state
            state[j] = 1.0 if s >= th[t] else -1.0
    return state.astype(np.float32)


# revision 53
# speedup vs baseline: 1.1263x; 1.1263x over previous
"""Trainium2 Bass kernel for nn_BoltzmannMachine: one sequential Gibbs sweep
over N=8192 units (order `perm`), distributed over 8 NeuronCores.

Algorithm (exact, validated vs the jax reference):
  sigmoid(s/T) >= u  <=>  s >= T*logit(u); thresholds are precomputed on
  device from u. Clamped units (half of them) never change and need no
  field evaluation, so only the NF=4096 free steps are processed: 8 row
  blocks of 512. Clamped COLUMNS contribute a purely static s0-signed
  row-sum folded into each block's margin. Within a block the decision
  bits satisfy a strictly-lower-triangular fixed point solved by Jacobi
  iteration on PE+DVE (R_ROUNDS covers the empirical worst case, with a
  convergence flag + retry ladder for safety). Block-start margins
  accumulate per-core partial sums over each core's 1024-column stripe
  (512 free + 512 clamped cols, premultiplied by s0 signs on the host -
  exact +-1 flips); an AllGather combines them.

Pipeline highlights:
  - Per-block AllGathers, each fired one tick ahead of use: block m's
    snapshot (taken at tick m-1 start) already contains the streamed
    flip-sign pieces for cols <= m-2, so only the H1 delta correction
    (col m-1, 16 matmuls) sits between a block's resolution and the
    next Jacobi solve. This keeps the replicated H-pack DMA at one
    level (7MB/core instead of 20MB under quad-grouped AGs) - the body
    is DMA-throughput-bound, so less traffic beats fewer collectives.
  - PSUM accumulation groups are never left open across other matmuls
    (the hardware semantics forbid interleaving).
  - Per-block Jacobi round counts follow ROUNDS_SCHED (empirical worst
    case + safety margin; a convergence flag + host margin-replay
    verification + retry ladder guarantee correctness for any input).
  - The streamed block-column pieces read a per-partition-contiguous
    host layout (wpiece) so every DMA run is >= 512B (full-rate).
  - V@h0 per block precomputed on PE ahead of time; each Jacobi round
    is 10 matmuls + one is_ge vector op against a negated threshold.
  - DMA load split across both HWDGE queues (SP + Activation); the
    collectives keep the Pool queue to themselves.

Host does data movement on w (permutation gather / re-layout / sign
flips) plus O(N) precompute; all O(N^2) FLOPs and the sequential
resolution run on device.
"""
import numpy as np

N = 8192
NF = 4096          # free steps (clamping_degree==0); harness input has 4096
B = 512
CORES = 8
F = B // 128
NBLK = NF // B     # 8 row blocks / free col blocks
RT = NF // 128     # 32 row tiles
SW = N // CORES    # 1024 stripe cols per core: 512 free + 512 clamped
BW = B // CORES    # 64 free cols per block per core
R_ROUNDS = 10

NPAIR = F * (F + 1) // 2


def _tile_order(vec):
    return np.ascontiguousarray(vec.reshape(RT, 128).T)


def _depth(m):
    """number of H corrections for block m under the AG grouping.

    Per-block AGs fire one tick ahead (block m's snapshot at tick m-1
    start already contains the streamed pieces for cols <= m-2), so only
    the H1 delta correction (col m-1) is ever needed.  This cuts the
    replicated H-pack DMA from 20MB to 7MB per core."""
    return 0 if m == 0 else 1


ROUNDS_SCHED = (6, 8, 7, 8, 11, 6, 6, 5)   # empirical per-block + safety 3


def _build_nc(R=R_ROUNDS, timing_no_cc=False, body_reps=1, sched=None,
              engines_v2=False, psh_early=True):
    import concourse.bacc as bacc
    import concourse.bass as bass
    import concourse.mybir as mybir
    from concourse.tile import TileContext

    f32 = mybir.dt.float32
    AO = mybir.AluOpType

    nc = bacc.Bacc("TRN2", target_bir_lowering=False, debug=False,
                   num_devices=CORES)

    wstripe = nc.declare_dram_parameter("wstripe", [NF, SW], f32,
                                        isOutput=False)
    wpiece = nc.declare_dram_parameter("wpiece", [128, NBLK * RT * BW], f32,
                                       isOutput=False)
    vpack = nc.declare_dram_parameter("vpack", [128, NBLK * NPAIR * 128], f32,
                                      isOutput=False)
    h1pack = nc.declare_dram_parameter("h1pack", [128, NBLK * F * F * 128],
                                       f32, isOutput=False)
    u_t = nc.declare_dram_parameter("u_t", [128, RT], f32, isOutput=False)
    q_t = nc.declare_dram_parameter("q_t", [128, RT], f32, isOutput=False)
    h0_t = nc.declare_dram_parameter("h0_t", [128, RT], f32, isOutput=False)
    s0_t = nc.declare_dram_parameter("s0_t", [128, RT], f32, isOutput=False)
    t_rep = nc.declare_dram_parameter("t_rep", [128, 1], f32, isOutput=False)
    out_d = nc.declare_dram_parameter("out_vals", [128, RT], f32,
                                      isOutput=True)
    flg_d = nc.declare_dram_parameter("out_flags", [128, NBLK], f32,
                                      isOutput=True)

    with TileContext(nc) as tc:
        with (
            tc.tile_pool(name="res", bufs=1) as res,
            tc.tile_pool(name="wbig", bufs=4) as wbig,
            tc.tile_pool(name="prod", bufs=2) as prodp,
            tc.tile_pool(name="pk", bufs=3) as pkp,
            tc.tile_pool(name="sm", bufs=3) as smp,
            tc.tile_pool(name="ytp", bufs=2) as ytpool,
            tc.tile_pool(name="ps", bufs=2, space=bass.MemorySpace.PSUM) as psp,
            tc.tile_pool(name="psv2", bufs=1,
                         space=bass.MemorySpace.PSUM) as psvp,
            tc.tile_pool(name="psh2", bufs=1,
                         space=bass.MemorySpace.PSUM) as pshp,
            tc.tile_pool(name="psvb", bufs=2,
                         space=bass.MemorySpace.PSUM) as psvbp,
            tc.tile_pool(name="cin", bufs=3, space="DRAM") as cin,
            tc.tile_pool(name="cout", bufs=3, space="DRAM") as cout,
        ):
            cid = nc.vector.partition_id()
            cid_be = nc.gpsimd.partition_id() if engines_v2 else cid
            # engine roles: DVE (`se`) runs the latency-critical small ops;
            # in v2 the big streaming reduces move to the Pool engine so
            # they never head-of-line-block a Jacobi round in the DVE
            # queue. ACT+SP keep the bulk DMA triggers.
            se = nc.vector
            be = nc.gpsimd if engines_v2 else nc.vector
            bgq = (nc.sync, nc.scalar)

            # ---------- resident tiles ----------
            acc = res.tile([128, RT], f32)
            th = res.tile([128, RT], f32)
            qt = res.tile([128, RT], f32)
            h0 = res.tile([128, RT], f32)
            s0t = res.tile([128, RT], f32)
            outv = res.tile([128, RT], f32)
            flags = res.tile([128, NBLK], f32)
            bits = res.tile([128, F], f32)
            mb0 = res.tile([128, F], f32)
            nm = res.tile([128, F], f32)
            bprev = res.tile([128, F], f32)
            trep = res.tile([128, 1], f32)
            ones1 = res.tile([1, 128], f32)
            fs = res.tile([128, F], f32)
            fsrow = res.tile([1, B], f32)
            d0 = res.tile([128, F], f32)
            d1 = res.tile([128, F], f32)
            d2 = res.tile([128, F], f32)
            d3 = res.tile([128, F], f32)
            d4 = res.tile([128, F], f32)
            d5 = res.tile([128, F], f32)
            dtiles = [d0, d1, d2, d3, d4, d5]

            for rep in range(body_reps):
                nc.vector.memset(acc[:, :], 0.0)
                nc.vector.memset(flags[:, :], 0.0)
                for dt_ in dtiles:
                    nc.vector.memset(dt_[:, :], 0.0)
                nc.vector.memset(ones1[:, :], 1.0)

                # ---------- load vectors ----------
                utile = smp.tile([128, RT], f32, tag="uload")
                nc.sync.dma_start(out=utile[:, :], in_=u_t[:, :])
                nc.scalar.dma_start(out=h0[:, :], in_=h0_t[:, :])
                nc.scalar.dma_start(out=s0t[:, :], in_=s0_t[:, :])
                nc.scalar.dma_start(out=qt[:, :], in_=q_t[:, :])
                nc.sync.dma_start(out=trep[:, :], in_=t_rep[:, :])

                # th = T * (ln(u) - ln(1-u))
                lu = smp.tile([128, RT], f32, tag="lu")
                om = smp.tile([128, RT], f32, tag="om")
                nc.scalar.activation(lu[:, :], utile[:, :],
                                     mybir.ActivationFunctionType.Ln)
                se.tensor_scalar(om[:, :], utile[:, :], -1.0, 1.0,
                                 AO.mult, AO.add)
                nc.scalar.activation(om[:, :], om[:, :],
                                     mybir.ActivationFunctionType.Ln)
                se.tensor_tensor(out=lu[:, :], in0=lu[:, :],
                                 in1=om[:, :], op=AO.subtract)
                se.tensor_scalar(th[:, :], lu[:, :], trep[:, 0:1],
                                 None, AO.mult)

                # ---------- helper: matvec contribution ----------
                qtoggle = [0]

                def piece(row_tile0, n_row_tiles, colL0, colW, vb_ap,
                          blk=None):
                    """acc[rows] += sum_cols wstripe(rows, cols) [* vb]"""
                    X = n_row_tiles
                    eng = bgq[qtoggle[0] % 2]
                    qtoggle[0] += 1
                    wt = wbig.tile([128, X * colW], f32, tag="wt")
                    if blk is not None:
                        # contiguous per-partition layout: >=512B runs
                        wpv = wpiece.ap().rearrange("p (k xt c) -> p k xt c",
                                                    k=NBLK, xt=RT)
                        xt0 = row_tile0
                        eng.dma_start(
                            out=wt[:, :].rearrange("p (xt c) -> p xt c", xt=X),
                            in_=wpv[:, blk, xt0:xt0 + X, :])
                    else:
                        wsv = wstripe.ap().rearrange("(xt p) c -> p xt c",
                                                     p=128)
                        eng.dma_start(
                            out=wt[:, :].rearrange("p (xt c) -> p xt c", xt=X),
                            in_=wsv[:, row_tile0:row_tile0 + X,
                                    colL0:colL0 + colW])
                    if vb_ap is not None:
                        pr = prodp.tile([128, X * colW], f32, tag="pr")
                        nc.vector.scalar_tensor_tensor(
                            out=pr[:, :].rearrange("p (xt c) -> p xt c", xt=X),
                            in0=wt[:, :].rearrange("p (xt c) -> p xt c", xt=X),
                            scalar=1.0,
                            in1=vb_ap.unsqueeze(1).to_broadcast(
                                (128, X, colW)),
                            op0=AO.mult, op1=AO.mult)
                        src = pr
                    else:
                        src = wt
                    red = smp.tile([128, X], f32, tag="red")
                    nc.vector.tensor_reduce(
                        out=red[:, :],
                        in_=src[:, :X * colW].rearrange("p (xt c) -> p xt c",
                                                        xt=X),
                        axis=mybir.AxisListType.X, op=AO.add)
                    be.tensor_tensor(
                        out=acc[:, row_tile0:row_tile0 + X],
                        in0=acc[:, row_tile0:row_tile0 + X],
                        in1=red[:, :], op=AO.add)

                def upper(m):
                    # static premultiplied s0 contribution (pure reduce):
                    # free columns >= m - depth(m) plus ALL clamped columns
                    # (the clamped 512 sit at stripe offset 512..1023, so
                    # the range [colL0, SW) covers both in one sweep)
                    colL0 = max(0, (m - _depth(m)) * BW)
                    c0 = colL0
                    while c0 < SW:
                        cw = min(512, SW - c0)
                        piece(F * m, F, c0, cw, None)
                        c0 += cw

                rg = [list(range(CORES))]
                outb = {}

                def trigger_ag_n(m, nb):
                    """AllGather for blocks {m .. m+nb-1}."""
                    ib = cin.tile([128, nb * F], f32, tag=f"ib{nb}", bufs=3)
                    ob = cout.tile([CORES * 128, nb * F], f32, tag=f"ob{nb}",
                                   bufs=3)
                    nc.sync.dma_start(out=ib[:, :],
                                      in_=acc[:, F * m:F * (m + nb)])
                    if timing_no_cc:
                        nc.sync.dma_start(out=ob[0:128, :], in_=ib[:, :])
                    else:
                        nc.gpsimd.collective_compute(
                            "AllGather", AO.bypass, replica_groups=rg,
                            ins=[ib[:, :].opt()], outs=[ob[:, :].opt()])
                    for i in range(nb):
                        outb[m + i] = (ob, i * F, nb)

                def qeng():
                    eng = bgq[qtoggle[0] % 2]
                    qtoggle[0] += 1
                    return eng

                def prefetch_packs(m):
                    vp = pkp.tile([128, NPAIR * 128], f32, tag="vp")
                    off = m * NPAIR * 128
                    qeng().dma_start(out=vp[:, :],
                                     in_=vpack[:, off:off + NPAIR * 128])
                    hsrc_d = {1: h1pack}
                    hps = []
                    off = m * F * F * 128
                    for k in range(1, _depth(m) + 1):
                        hk = pkp.tile([128, F * F * 128], f32, tag=f"hp{k}")
                        qeng().dma_start(
                            out=hk[:, :],
                            in_=hsrc_d[k][:, off:off + F * F * 128])
                        hps.append(hk)
                    return (vp, hps)

                def compute_vh0(m, vp):
                    ps = psvp.tile([128, F], f32, tag="vh0")
                    for mc in range(F):
                        for kc in range(mc + 1):
                            poff = (mc * (mc + 1) // 2 + kc) * 128
                            nc.tensor.matmul(
                                ps[:, mc:mc + 1],
                                vp[:, poff:poff + 128],
                                h0[:, F * m + kc:F * m + kc + 1],
                                start=(kc == 0), stop=(kc == mc))
                    sb_t = smp.tile([128, F], f32, tag="vh0s", bufs=3)
                    nc.vector.tensor_copy(sb_t[:, :], ps[:, :])
                    return sb_t

                # ---------- prefill ----------
                upper(0)
                upper(1)
                trigger_ag_n(0, 2)
                upper(2)
                upper(3)
                upper(4)
                upper(5)

                packs = {0: prefetch_packs(0), 1: prefetch_packs(1)}
                vh0ps = {0: compute_vh0(0, packs[0][0]),
                         1: compute_vh0(1, packs[1][0])}

                # ---------- main tick loop ----------
                pshpend = {}

                def _psh_mms(tilep, hs, start0):
                    for mc in range(F):
                        for hi, (h, d) in enumerate(hs):
                            for kc in range(F):
                                nc.tensor.matmul(
                                    tilep[:, mc:mc + 1],
                                    h[:, (kc * F + mc) * 128:
                                      (kc * F + mc) * 128 + 128],
                                    d[:, kc:kc + 1],
                                    start=(start0 and hi == 0 and kc == 0),
                                    stop=(hi == len(hs) - 1 and
                                          kc == F - 1))

                def prep_psh_early(m):
                    """levels >=2 for block m as a CLOSED accumulation group
                    (their deltas are final before tick m-1 ends, so the PE
                    runs this during tick m-1's tail, off the critical
                    path). PSUM groups must not interleave with other
                    matmuls, so H1 goes to a second tile at tick m."""
                    hps = packs[m][1]
                    tile_e = None
                    if psh_early and len(hps) >= 2:
                        tile_e = pshp.tile([128, F], f32, tag="pshE")
                        hs = [(hps[k - 1], dtiles[(m - k) % 6])
                              for k in range(2, len(hps) + 1)]
                        _psh_mms(tile_e, hs, True)
                    pshpend[m] = tile_e

                def finish_psh(m):
                    """H1 (deltas of block m-1); with psh_early off, all
                    levels run here (old style)."""
                    tile_e = pshpend.pop(m)
                    hps = packs[m][1]
                    levels = [1] if tile_e is not None else \
                        list(range(1, len(hps) + 1))
                    hs = [(hps[k - 1], dtiles[(m - k) % 6]) for k in levels]
                    tile1 = pshp.tile([128, F], f32, tag="psh1")
                    _psh_mms(tile1, hs, True)
                    return (tile1, tile_e)

                for j in range(NBLK):
                    dcur = dtiles[j % 6]

                    # 0) fire block j+1's AG: its snapshot already holds the
                    #    streamed pieces for cols <= j-1 (emitted in earlier
                    #    tick tails), so only H1 remains for tick j+1
                    if 1 <= j <= NBLK - 2:
                        trigger_ag_n(j + 1, 1)

                    # 1) close block j's H correction with the H1 matmuls
                    psh = finish_psh(j) if j >= 1 else None

                    # 3) gather AG result, reduce over cores
                    ob, ofs, nbw = outb[j]
                    if j == 0 or j >= 2:
                        yt2 = ytpool.tile([128, nbw * F * CORES], f32,
                                          tag="yt2")
                        nc.sync.dma_start(
                            out=yt2[:, :].rearrange("p (f c) -> p f c",
                                                    c=CORES),
                            in_=ob[:, :].rearrange("(c p) f -> p f c", p=128))
                        ytcache = yt2
                    yv = smp.tile([128, F], f32, tag="yv")
                    se.tensor_reduce(
                        out=yv[:, :],
                        in_=ytcache[:, :].rearrange(
                            "p (f c) -> p f c", c=CORES)[:, ofs:ofs + F, :],
                        axis=mybir.AxisListType.X, op=AO.add)

                    # 4) mb0 = yv - th (+ psh); bits0 = [mb0 >= 0]
                    se.tensor_tensor(out=mb0[:, :], in0=yv[:, :],
                                     in1=th[:, F * j:F * (j + 1)],
                                     op=AO.subtract)
                    if psh is not None:
                        tile1, tile_e = psh
                        if tile_e is not None:
                            se.tensor_tensor(out=mb0[:, :], in0=mb0[:, :],
                                             in1=tile_e[:, :], op=AO.add)
                        se.tensor_tensor(out=mb0[:, :], in0=mb0[:, :],
                                         in1=tile1[:, :], op=AO.add)
                    se.tensor_scalar(bits[:, :], mb0[:, :], 0.0, None,
                                     AO.is_ge)
                    se.scalar_tensor_tensor(
                        out=nm[:, :], in0=vh0ps[j][:, :], scalar=-1.0,
                        in1=mb0[:, :], op0=AO.mult, op1=AO.subtract)

                    # 5) fixed-point rounds
                    vp = packs[j][0]
                    Rj = sched[j] if sched is not None else R
                    for r in range(1, Rj):
                        if r == Rj - 1:
                            se.tensor_copy(bprev[:, :], bits[:, :])
                        psv = psp.tile([128, F], f32, tag="psv")
                        for mc in range(F):
                            for kc in range(mc + 1):
                                poff = (mc * (mc + 1) // 2 + kc) * 128
                                nc.tensor.matmul(
                                    psv[:, mc:mc + 1],
                                    vp[:, poff:poff + 128],
                                    bits[:, kc:kc + 1],
                                    start=(kc == 0), stop=(kc == mc))
                        se.tensor_tensor(out=bits[:, :],
                                         in0=psv[:, :],
                                         in1=nm[:, :], op=AO.is_ge)

                    # 6) delta first (it unblocks tick j+1's H1 and the
                    #    flip-sign pieces), outv, then flags off-path
                    # delta = 2*bits - (1+s0)
                    se.scalar_tensor_tensor(
                        out=dcur[:, :], in0=bits[:, :], scalar=2.0,
                        in1=qt[:, F * j:F * (j + 1)],
                        op0=AO.mult, op1=AO.subtract)
                    se.tensor_tensor(out=outv[:, F * j:F * (j + 1)],
                                     in0=s0t[:, F * j:F * (j + 1)],
                                     in1=dcur[:, :], op=AO.add)

                    # ---- tail: prepare tick j+1 ----
                    # 7) early H levels (>=2) for block j+1 run on PE now
                    if j + 1 < NBLK:
                        prep_psh_early(j + 1)

                    # 8) flip-sign pieces for col j onto rows whose AG
                    #    snapshot is still in the future (blocks >= j+2)
                    sb = j + 2
                    X = RT - F * sb
                    if X > 0:
                        se.tensor_tensor(
                            out=fs[:, :], in0=dcur[:, :],
                            in1=s0t[:, F * j:F * (j + 1)], op=AO.mult)
                        se.tensor_scalar(fs[:, :], fs[:, :], 1.0,
                                         None, AO.add)
                        nc.scalar.dma_start(
                            out=fsrow[0:1, :].rearrange("a (p f) -> a p f",
                                                        p=128),
                            in_=fs[:, :])
                        vbps = psvbp.tile([128, B], f32, tag="vb")
                        nc.tensor.matmul(vbps[:, :], ones1[:, :],
                                         fsrow[:, :], start=True, stop=True)
                        piece(F * sb, X, j * BW, BW,
                              vbps[:, bass.ts(cid, BW)], blk=j)

                    # 9) flags (convergence check, off the critical path)
                    dtmp = smp.tile([128, F], f32, tag="dtmp")
                    se.tensor_tensor(out=dtmp[:, :], in0=bits[:, :],
                                     in1=bprev[:, :], op=AO.subtract)
                    nc.vector.tensor_reduce(out=flags[:, j:j + 1],
                                            in_=dtmp[:, :],
                                            axis=mybir.AxisListType.X,
                                            op=AO.add,
                                            apply_absolute_value=True)

                    # 10) background: upper(j+6) (prefill issued 0..5)
                    if j + 6 < NBLK:
                        upper(j + 6)

                    # 11) background: prefetch packs + Vh0 two ticks ahead
                    if j + 2 < NBLK and (j + 2) not in packs:
                        packs[j + 2] = prefetch_packs(j + 2)
                        vh0ps[j + 2] = compute_vh0(j + 2, packs[j + 2][0])
                    packs.pop(j - 1, None)
                    vh0ps.pop(j - 1, None)

                nc.sync.dma_start(out=out_d[:, :], in_=outv[:, :])
                nc.sync.dma_start(out=flg_d[:, :], in_=flags[:, :])

    nc.compile()
    return nc


def _host_prep(w, initial_state, u, T, clamping_degree, perm):
    w = np.asarray(w, dtype=np.float32)
    s0 = np.asarray(initial_state, dtype=np.float32)
    u = np.asarray(u, dtype=np.float32)
    cd = np.asarray(clamping_degree)
    perm = np.asarray(perm).astype(np.int64)
    Tf = np.float32(T)

    free_step = cd[perm] == 0
    js = np.nonzero(free_step)[0]            # free step indices, in order
    cs = np.nonzero(~free_step)[0]           # clamped step indices
    jj = perm[js]                            # free unit ids, visit order
    ccu = perm[cs]                           # clamped unit ids

    s0f = s0[jj]
    uf = u[js]
    h0 = (-0.5 * (1.0 + s0f)).astype(np.float32)

    wr = w[jj]                               # [NF, N] rows in visit order

    jf, pt, ff = np.meshgrid(np.arange(NBLK), np.arange(16), np.arange(F),
                             indexing="ij")

    def stripe_free(c):
        # free-column visit indices owned by core c, stripe order
        return (jf * B + 128 * ff + 16 * c + pt).reshape(-1)

    # intra/inter block packs (free x free couplings)
    wff = w[np.ix_(jj, jj)]
    vpack = np.zeros((128, NBLK * NPAIR * 128), dtype=np.float32)
    h1pack = np.zeros((128, NBLK * F * F * 128), dtype=np.float32)
    tril = np.tril(np.ones((B, B), dtype=np.float32), -1)
    for j in range(NBLK):
        blk = wff[j * B:(j + 1) * B, j * B:(j + 1) * B]
        V = (blk * tril) * 2.0               # f2 = 2 (all free)
        hsrc = {}
        for k in range(1, _depth(j) + 1):
            hsrc[k] = wff[j * B:(j + 1) * B, (j - k) * B:(j - k + 1) * B]
        packs_ = {1: h1pack}
        for mc in range(F):
            for kc in range(F):
                colbase = (j * F * F + kc * F + mc) * 128
                for lvl, Hm in hsrc.items():
                    packs_[lvl][:, colbase:colbase + 128] = \
                        Hm[mc * 128:(mc + 1) * 128,
                           kc * 128:(kc + 1) * 128].T
            for kc in range(mc + 1):
                vbase = (j * NPAIR + mc * (mc + 1) // 2 + kc) * 128
                vpack[:, vbase:vbase + 128] = V[mc * 128:(mc + 1) * 128,
                                                kc * 128:(kc + 1) * 128].T

    common = {
        "vpack": vpack,
        "h1pack": h1pack,
        "u_t": _tile_order(uf),
        "q_t": _tile_order((1.0 + s0f).astype(np.float32)),
        "h0_t": _tile_order(h0),
        "s0_t": _tile_order(s0f),
        "t_rep": np.full((128, 1), Tf, dtype=np.float32),
    }
    in_maps = []
    ncl = len(ccu)
    for c in range(CORES):
        sf = stripe_free(c)
        # clamped columns owned by core c (even 512-way split)
        scl = ccu[(np.arange(ncl) % CORES) == c]
        # stripe = [512 free cols (visit order) | 512 clamped cols],
        # premultiplied by their s0 signs (exact +-1 flips)
        cols_units = np.concatenate([jj[sf], scl])
        sgn = s0[cols_units][None, :]
        wsm = np.ascontiguousarray(wr[:, cols_units] * sgn)
        m = dict(common)
        m["wstripe"] = wsm
        # per-partition-contiguous block-column layout for the bulk pieces
        # (free half only)
        m["wpiece"] = np.ascontiguousarray(
            wsm[:, :NBLK * BW].reshape(RT, 128, NBLK, BW)
            .transpose(1, 2, 0, 3).reshape(128, NBLK * RT * BW))
        in_maps.append(m)
    meta = {"perm": perm, "s0": s0, "jj": jj, "wr": wr,
            "th64": np.float64(T) * (np.log(np.float64(uf)) -
                                     np.log1p(-np.float64(uf)))}
    return in_maps, meta


def _verify_device_result(vals_pp, meta, tol=1e-3):
    """Replay the blockwise margins on host and confirm every device
    decision.  Decisions inside +-tol of the threshold are accepted (that
    is the irreducible fp32 ambiguity band); anything farther off means a
    corrupted run (e.g. a DMA race) and triggers a retry/fallback."""
    jj = meta["jj"]
    wr = meta["wr"]
    th = meta["th64"]
    if not np.isfinite(vals_pp).all() or \
            np.abs(np.abs(vals_pp) - 1.0).max() != 0.0:
        return False
    state = np.array(meta["s0"], dtype=np.float32, copy=True)
    for b in range(NBLK):
        sl = slice(b * B, (b + 1) * B)
        dots = (wr[sl].astype(np.float64) @ np.float64(state))
        margin = dots - th[sl]
        want = np.where(margin >= 0, np.float32(1.0), np.float32(-1.0))
        got = vals_pp[sl]
        bad = (want != got) & (np.abs(margin) > tol)
        if bad.any():
            return False
        state[jj[sl]] = got
    return True


_NC_CACHE = {}
LAST_RESULTS = None  # BassKernelResults of the final device run (for test.py)


def _get_nc(key, **kw):
    if key not in _NC_CACHE:
        _NC_CACHE[key] = _build_nc(**kw)
    return _NC_CACHE[key]


def default_nc(body_reps=1):
    return _get_nc(("sched", ROUNDS_SCHED, body_reps),
                   sched=ROUNDS_SCHED, body_reps=body_reps,
                   engines_v2=False)


def kernel(**inputs):
    global LAST_RESULTS
    from concourse.bass_utils import run_bass_kernel_spmd
    import os

    w = inputs["w"]
    perm = np.asarray(inputs["perm"]).astype(np.int64)
    cd = np.asarray(inputs["clamping_degree"])
    # fast path requires a true permutation and the expected free count
    is_perm = (np.sort(perm) == np.arange(N)).all()
    if not is_perm or int((cd[perm] == 0).sum()) != NF:
        return _reference_fallback(**inputs)

    in_maps, meta = _host_prep(**inputs)
    trace = os.environ.get("KERNEL_TRACE", "0") == "1"

    # retry ladder: per-block schedule first, then uniform deep rounds.
    # every candidate result is verified on the host (blockwise margin
    # replay) before being accepted, so a flaky run can never escape.
    ok = False
    for attempt in ("sched", 16, 64):
        if attempt == "sched":
            nc = default_nc()
        else:
            nc = _get_nc(("uniform", attempt), R=attempt)
        res = run_bass_kernel_spmd(nc, in_maps, core_ids=list(range(CORES)),
                                   trace=trace)
        LAST_RESULTS = res
        vals_t = res.results[0]["out_vals"]       # [128, RT] tile layout
        flags = res.results[0]["out_flags"]
        vals_pp = vals_t.T.reshape(-1)            # [NF] free-step order
        if float(np.abs(flags).sum()) == 0.0 and \
                _verify_device_result(vals_pp, meta):
            ok = True
            break
    if not ok:
        return _reference_fallback(**inputs)
    out = np.array(meta["s0"], dtype=np.float32, copy=True)
    out[meta["jj"]] = vals_pp
    return out


def _reference_fallback(w, initial_state, u, T, clamping_degree, perm):
    """Generic (repeat-tolerant) path: exact sequential numpy replay.

    Only used when `perm` is not a permutation or the clamping pattern
    differs from the expected harness inputs.
    """
    state = np.asarray(initial_state, dtype=np.float64).copy()
    w64 = np.asarray(w, dtype=np.float64)
    free = (np.asarray(clamping_degree) == 0)
    th = float(T) * (np.log(np.float64(u)) - np.log1p(-np.float64(u)))
    for t in range(len(perm)):
        j = int(perm[t])
        if free[j]:
            s = w64[j] @ state
            state[j] = 1.0 if s >= th[t] else -1.0
    return state.astype(np.float32)
